# revision 1
# baseline (speedup 1.0000x reference)
"""DeepAR (2-layer LSTM, H=512) Trainium2 Bass kernel, 8-core data-parallel.

Model (see reference): x = concat(x_cont, emb0[cat0], emb1[cat1]) [B,T,56]
  -> LSTM(512) -> LSTM(512) -> mu = h@Wmu+bmu ; sigma = softplus(h@Wsig+bsig)

Sharding: batch B=256 split across 8 cores (32 rows each); params replicated.

Per-core device program (matmul operands + elementwise bf16, psum fp32):
  - embeddings: per-128-row-tile indirect DMA gathers (multi-index indirect
    DMA corrupts SBUF on HW) assembled with x_cont + a ones row, then
    PE-transposed into x^T [57, (t,b)] bf16
  - L1 scan, per gate-slice n (n-outer so ACT/DVE overlap later matmuls):
    gates_n [32,512] = [x^T_t;1] @ [Wk1;b1]_n + sum_c h1T_c @ Wr1_c,n
  - h1 PE-transposed each step into h1T history [128, KC, T, 32] bf16
  - xz2 m-tiles (= h1 @ Wk2, full-M matmuls) interleaved into the L1 scan
    4 steps behind the recurrence to fill PE gaps; b2 is folded into the
    L2 input chunk via an [I32;ones] stationary against a double-buffered
    xz station whose row 32 holds b2
  - L2 scan: gates = [I;1] @ [xz2_t;b2] + sum_c h2T_c @ Wr2_c, with the
    output-head slices interleaved every 16 steps
  - head: mu/sigma^T [1, 512] = sum_c WmsT_c @ h2T_hist; mu += bmu (DVE),
    sigma = Ln(Exp(x + bsig) + 1)  (no Softplus ACT table on this build)
"""

import numpy as np
import ml_dtypes

import concourse.bass as bass
import concourse.mybir as mybir
import concourse.tile as tile
from concourse import bacc
from concourse.masks import make_identity

F32 = mybir.dt.float32
BF16 = mybir.dt.bfloat16
I16 = mybir.dt.int16
I32 = mybir.dt.int32

B, T, F = 256, 192, 8
CARD0, CARD1 = 1000, 100
E0, E1 = 32, 16
H = 512
DIN = F + E0 + E1          # 56
G4 = 4 * H                 # 2048
NC_N = 8                   # cores
BSH = B // NC_N            # 32 batch rows per core
R = T * BSH                # 6144 (t,b)-ordered rows per core
KC = H // 128              # 4 recurrent K-chunks
NS = 512                   # matmul free-dim slice
NN = G4 // NS              # 4 N-slices
A = mybir.ActivationFunctionType


def _lstm_scan(nc, tc, pools, layer, xsrc, w_sb, hist, ident_f32, ident_bf16,
               post_step=None):
    """One LSTM layer scan over T steps.

    layer 1: xsrc = xT sbuf tile [64, R] (rows 0..56 = x^T plus ones row),
             w_sb [128, 5, G4] (chunk0 = [Wk1;b1], chunks1-4 = Wr1)
    layer 2: xsrc = (xz2_dram, ld_pool); w_sb [128, 4, G4] = Wr2 chunks
    hist: persistent sbuf tile [128, KC, T, BSH] bf16 written with h^T chunks.
    """
    ew = pools["ew"]
    ps_gates = pools["ps_gates"]
    ps_tr = pools["ps_tr"]

    c_state = pools["state"].tile([BSH, H], BF16)

    for t in range(T):
        # ---- per-gate matmul accumulation: n-outer, chunks inner ----
        # gate n finishes early so ACT/DVE overlap the remaining matmuls
        if layer == 1:
            lhsT0 = xsrc[0:DIN + 1, t * BSH:(t + 1) * BSH]   # [57, 32]
            rhs0 = w_sb[0:DIN + 1, 0, :]                     # [57, G4]
        else:
            # station rows 0..31 <- xz_t (parity double-buffered); row 32 = b2
            xz_dram, station, iones = xsrc
            par = t % 2
            nc.sync.dma_start(out=station[0:BSH, par, :],
                              in_=xz_dram[t * BSH:(t + 1) * BSH, :])
            lhsT0 = iones[0:BSH + 1, 0:BSH]
            rhs0 = station[0:BSH + 1, par, :]
        woff = 1 if layer == 1 else 0
        gate_ps = []
        for n in range(NN):
            g_ps = ps_gates.tile([BSH, NS], F32, tag="g")
            gate_ps.append(g_ps)
            nsl = slice(n * NS, (n + 1) * NS)
            nc.tensor.matmul(g_ps[:], lhsT0, rhs0[:, nsl],
                             start=True, stop=(t == 0))
            if t > 0:
                for c in range(KC):
                    nc.tensor.matmul(g_ps[:], hist[:, c, t - 1, :],
                                     w_sb[:, woff + c, nsl],
                                     start=False, stop=(c == KC - 1))

        # ---- gate nonlinearities (order i, f, g, o), bf16 for DVE 2x ----
        sig_i = ew.tile([BSH, H], BF16)
        sig_f = ew.tile([BSH, H], BF16)
        tan_g = ew.tile([BSH, H], BF16)
        sig_o = ew.tile([BSH, H], BF16)
        nc.scalar.activation(sig_i[:], gate_ps[0][:], A.Sigmoid)
        nc.scalar.activation(sig_f[:], gate_ps[1][:], A.Sigmoid)
        nc.scalar.activation(tan_g[:], gate_ps[2][:], A.Tanh)
        nc.scalar.activation(sig_o[:], gate_ps[3][:], A.Sigmoid)

        # ---- cell/state update ----
        ig = ew.tile([BSH, H], BF16)
        nc.vector.tensor_mul(ig[:], sig_i[:], tan_g[:])
        if t == 0:
            nc.vector.tensor_copy(c_state[:], ig[:])
        else:
            fc = ew.tile([BSH, H], BF16)
            nc.vector.tensor_mul(fc[:], sig_f[:], c_state[:])
            nc.vector.tensor_add(c_state[:], fc[:], ig[:])
        # ---- h^T = sig_o^T * tanh(c^T), all in transposed space:
        # sig_o and c transposes overlap the remaining matmuls; the tail is
        # just ACT tanh on c^T plus one DVE mul that writes hist in place
        ps_so = ps_tr.tile([128, KC * BSH], BF16, tag="so")
        ps_tc = ps_tr.tile([128, KC * BSH], BF16, tag="tc")
        for c in range(KC):
            nc.tensor.transpose(ps_so[:, c * BSH:(c + 1) * BSH],
                                sig_o[:, c * 128:(c + 1) * 128],
                                ident_bf16[0:BSH, 0:BSH])
        soT = ew.tile([128, KC * BSH], BF16)
        nc.vector.tensor_copy(soT[:], ps_so[:])
        for c in range(KC):
            nc.tensor.transpose(ps_tc[:, c * BSH:(c + 1) * BSH],
                                c_state[:, c * 128:(c + 1) * 128],
                                ident_bf16[0:BSH, 0:BSH])
        tan_cT = ew.tile([128, KC * BSH], BF16)
        nc.scalar.activation(tan_cT[:], ps_tc[:], A.Tanh)
        hview = bass.AP(tensor=hist.tensor, offset=hist.offset + t * BSH,
                        ap=[list(hist.ap[0]), [T * BSH, KC], [1, BSH]])
        nc.vector.tensor_mul(hview, soT[:], tan_cT[:])

        if post_step is not None:
            post_step(t)


_NC_CACHE = {}


def build_nc(upto="all"):
    if upto in _NC_CACHE:
        return _NC_CACHE[upto]
    from contextlib import ExitStack
    nc = bacc.Bacc("TRN2", num_devices=NC_N)

    # ---------------- DRAM I/O ----------------
    idx0_d = nc.dram_tensor("idx0", [128, R // 128], I32, kind="ExternalInput")
    idx1_d = nc.dram_tensor("idx1", [128, R // 128], I32, kind="ExternalInput")
    e0t_d = nc.dram_tensor("e0tab", [CARD0, E0], F32, kind="ExternalInput")
    e1t_d = nc.dram_tensor("e1tab", [CARD1, E1], F32, kind="ExternalInput")
    xcr_d = nc.dram_tensor("xcr", [128, R // 128, F], F32, kind="ExternalInput")
    w1_d = nc.dram_tensor("w1", [128, 1 + KC, G4], BF16, kind="ExternalInput")
    wk2_d = nc.dram_tensor("wk2", [128, KC, G4], BF16, kind="ExternalInput")
    w2_d = nc.dram_tensor("w2", [128, KC, G4], BF16, kind="ExternalInput")
    b2_d = nc.dram_tensor("b2v", [1, G4], F32, kind="ExternalInput")
    wms_d = nc.dram_tensor("wms", [128, KC, 2], BF16, kind="ExternalInput")
    bms_d = nc.dram_tensor("bms", [1, 2], F32, kind="ExternalInput")

    mu_d = nc.dram_tensor("mu", [BSH, T], F32, kind="ExternalOutput")
    sg_d = nc.dram_tensor("sigma", [BSH, T], F32, kind="ExternalOutput")
    dbg_d = nc.dram_tensor("dbg", [64, R], F32, kind="ExternalOutput") \
        if upto != "all" else None

    xz2_d = nc.dram_tensor("xz2scratch", [R, G4], BF16)  # internal scratch

    _build_body(nc, upto, locals())
    nc.compile()
    _NC_CACHE[upto] = nc
    return nc


def _build_body(nc, upto, env):
    from contextlib import ExitStack
    idx0_d = env["idx0_d"]; idx1_d = env["idx1_d"]; xcr_d = env["xcr_d"]
    e0t_d = env["e0t_d"]; e1t_d = env["e1t_d"]; w1_d = env["w1_d"]
    MT = R // 128
    wk2_d = env["wk2_d"]; w2_d = env["w2_d"]; b2_d = env["b2_d"]
    wms_d = env["wms_d"]; bms_d = env["bms_d"]; mu_d = env["mu_d"]
    sg_d = env["sg_d"]; xz2_d = env["xz2_d"]; dbg_d = env["dbg_d"]
    with tile.TileContext(nc) as tc, ExitStack() as top:  # noqa: SIM117
        singles = top.enter_context(tc.tile_pool(name="singles", bufs=1))

        # ---------------- constants / weights to SBUF ----------------
        wk2_sb = singles.tile([128, KC, G4], BF16)
        nc.sync.dma_start(out=wk2_sb[:], in_=wk2_d[:])
        w2_sb = singles.tile([128, KC, G4], BF16)
        nc.sync.dma_start(out=w2_sb[:], in_=w2_d[:])
        wms_sb = singles.tile([128, KC, 2], BF16)
        nc.sync.dma_start(out=wms_sb[:], in_=wms_d[:])
        bms_sb = singles.tile([1, 2], F32)
        nc.sync.dma_start(out=bms_sb[:], in_=bms_d[:])

        ident_f32 = singles.tile([128, 128], F32)
        make_identity(nc, ident_f32[:])
        ident_bf16 = singles.tile([128, 128], BF16)
        nc.vector.tensor_copy(ident_bf16[:], ident_f32[:])

        h1T = singles.tile([128, KC, T, BSH], BF16)
        h2T = singles.tile([128, KC, T, BSH], BF16)

        # ------- phases 1+2 share a pool so xT/w1 free before phase 4 -------
        p12 = ExitStack()
        xtp = p12.enter_context(tc.tile_pool(name="xtp", bufs=1))
        w1_sb = xtp.tile([128, 1 + KC, G4], BF16)
        nc.sync.dma_start(out=w1_sb[:], in_=w1_d[:])
        xT = xtp.tile([64, R], BF16)   # rows: 0-31 e0, 32-47 e1, 48-55 xc, 56 ones

        # ---------------- phase 1: build x^T ----------------
        with tc.tile_pool(name="gather", bufs=1) as gp, \
                tc.tile_pool(name="gtr", bufs=2, space="PSUM") as ptr:
            idx0_sb = gp.tile([128, MT], I32)
            nc.sync.dma_start(out=idx0_sb[:], in_=idx0_d[:])
            idx1_sb = gp.tile([128, MT], I32)
            nc.sync.dma_start(out=idx1_sb[:], in_=idx1_d[:])

            # assembled rows: [p, m, 64] = [e0 | e1 | xc | ones(+pad)]
            # NOTE: multi-index indirect DMA is broken on HW (stomps memory);
            # one gather per 128-row tile, single idx column each.
            asm = gp.tile([128, MT, 64], F32)
            nc.vector.memset(asm[:], 1.0)
            xcb = gp.tile([128, MT, F], F32)
            nc.sync.dma_start(out=xcb[:], in_=xcr_d[:])
            nc.vector.tensor_copy(asm[:, :, E0 + E1:DIN], xcb[:])
            for m in range(MT):
                nc.gpsimd.indirect_dma_start(
                    out=asm[:, m, 0:E0], out_offset=None, in_=e0t_d[:],
                    in_offset=bass.IndirectOffsetOnAxis(
                        ap=idx0_sb[:, m:m + 1], axis=0))
                nc.gpsimd.indirect_dma_start(
                    out=asm[:, m, E0:E0 + E1], out_offset=None, in_=e1t_d[:],
                    in_offset=bass.IndirectOffsetOnAxis(
                        ap=idx1_sb[:, m:m + 1], axis=0))
            for m in range(MT):
                ps = ptr.tile([64, 128], F32)
                nc.tensor.transpose(ps[:], asm[:, m, :], ident_f32[:])
                nc.vector.tensor_copy(xT[:, 128 * m:128 * (m + 1)], ps[:])

        if upto == "xT":
            with tc.tile_pool(name="dbgp", bufs=1) as dp:
                dbg_sb = dp.tile([64, R], F32)
                nc.vector.tensor_copy(dbg_sb[:], xT[:])
                nc.sync.dma_start(out=dbg_d[:], in_=dbg_sb[:])
            return
        # ---------------- phase 2: L1 scan + interleaved xz2 m-tiles ----------
        with ExitStack() as ph:
            pools = {
                "ew": ph.enter_context(tc.tile_pool(name="ew1", bufs=2)),
                "state": ph.enter_context(tc.tile_pool(name="st1", bufs=1)),
                "ps_gates": ph.enter_context(
                    tc.tile_pool(name="psg1", bufs=4, space="PSUM")),
                "ps_tr": ph.enter_context(
                    tc.tile_pool(name="pst1", bufs=1, space="PSUM")),
            }
            psxz = ph.enter_context(tc.tile_pool(name="psxz", bufs=2, space="PSUM"))
            xzs = ph.enter_context(tc.tile_pool(name="xzs", bufs=3))

            def xz2_tile(t):
                # after step t = 4m+3, rows for m-tile m are complete
                if (t + 1) % 4 != 0:
                    return
                m = (t + 1) // 4 - 1
                for n in range(NN):
                    nsl = slice(n * NS, (n + 1) * NS)
                    ps = psxz.tile([128, NS], F32, tag="xz")
                    for c in range(KC):
                        lhsT = h1T[:, c, 4 * m:4 * (m + 1), :]
                        nc.tensor.matmul(ps[:], lhsT, wk2_sb[:, c, nsl],
                                         start=(c == 0), stop=(c == KC - 1))
                    xz_sb = xzs.tile([128, NS], BF16, tag="xzs")
                    nc.vector.tensor_copy(xz_sb[:], ps[:])
                    nc.sync.dma_start(
                        out=xz2_d[128 * m:128 * (m + 1), nsl], in_=xz_sb[:])

            _lstm_scan(nc, tc, pools, 1, xT, w1_sb, h1T, ident_f32, ident_bf16,
                       post_step=xz2_tile)
        p12.close()

        # -------- phase 4: L2 scan + interleaved head slices ----------------
        with ExitStack() as ph:
            pools = {
                "ew": ph.enter_context(tc.tile_pool(name="ew2", bufs=2)),
                "state": ph.enter_context(tc.tile_pool(name="st2", bufs=1)),
                "ps_gates": ph.enter_context(
                    tc.tile_pool(name="psg2", bufs=4, space="PSUM")),
                "ps_tr": ph.enter_context(
                    tc.tile_pool(name="pst2", bufs=1, space="PSUM")),
            }
            psh = ph.enter_context(tc.tile_pool(name="psh", bufs=1, space="PSUM"))
            hew = ph.enter_context(tc.tile_pool(name="hew", bufs=2))
            stp = ph.enter_context(tc.tile_pool(name="stp", bufs=1))
            # xz station: rows 0..31 xz (parity-double-buffered), row 32 = b2
            station = stp.tile([64, 2, G4], BF16)
            b2row = bass.AP(tensor=b2_d[:].tensor, offset=0,
                            ap=[[0, 1], [0, 2], [1, G4]])
            nc.gpsimd.dma_start(out=station[BSH:BSH + 1, :, :], in_=b2row)
            # [I32; ones-row] stationary for the xz+b2 chunk
            iones = stp.tile([64, BSH], BF16)
            nc.vector.memset(iones[0:64, :], 0.0)
            nc.vector.tensor_copy(iones[0:BSH, :], ident_bf16[0:BSH, 0:BSH])
            nc.vector.memset(iones[BSH:BSH + 1, :], 1.0)
            TSL = NS // BSH  # 16 timesteps per head slice

            def head_slice(t):
                if (t + 1) % TSL != 0:
                    return
                n = (t + 1) // TSL - 1
                ps_mu = psh.tile([1, NS], F32, tag="hm")
                ps_sg = psh.tile([1, NS], F32, tag="hs")
                for c in range(KC):
                    rhs = h2T[:, c, n * TSL:(n + 1) * TSL, :]
                    nc.tensor.matmul(ps_mu[:], wms_sb[:, c, 0:1], rhs,
                                     start=(c == 0), stop=(c == KC - 1))
                    nc.tensor.matmul(ps_sg[:], wms_sb[:, c, 1:2], rhs,
                                     start=(c == 0), stop=(c == KC - 1))
                mu_sl = hew.tile([1, NS], F32)
                nc.vector.tensor_scalar_add(mu_sl[:], ps_mu[:], bms_sb[0:1, 0:1])
                ex = hew.tile([1, NS], F32)
                sg_sl = hew.tile([1, NS], F32)
                nc.scalar.activation(ex[:], ps_sg[:], A.Exp, bias=bms_sb[0:1, 1:2])
                nc.scalar.activation(sg_sl[:], ex[:], A.Ln, bias=1.0)
                mu_view = bass.AP(tensor=mu_d[:].tensor, offset=n * TSL,
                                  ap=[[0, 1], [1, TSL], [T, BSH]])
                nc.sync.dma_start(out=mu_view, in_=mu_sl[:])
                sg_view = bass.AP(tensor=sg_d[:].tensor, offset=n * TSL,
                                  ap=[[0, 1], [1, TSL], [T, BSH]])
                nc.sync.dma_start(out=sg_view, in_=sg_sl[:])

            _lstm_scan(nc, tc, pools, 2, (xz2_d, station, iones), w2_sb, h2T,
                       ident_f32, ident_bf16, post_step=head_slice)

    return nc


def _marshal(inputs):
    """Host-side shard/layout marshalling (no compute beyond dtype cast/pad)."""
    bf = ml_dtypes.bfloat16
    xc = np.ascontiguousarray(np.asarray(inputs["x_cont"], np.float32))
    cat0 = np.asarray(inputs["cat0"]).astype(np.int32)
    cat1 = np.asarray(inputs["cat1"]).astype(np.int32)
    emb0 = np.asarray(inputs["emb0"], np.float32)
    emb1 = np.asarray(inputs["emb1"], np.float32)
    Wk1 = np.asarray(inputs["Wk1"], np.float32)
    Wr1 = np.asarray(inputs["Wr1"], np.float32)
    b1 = np.asarray(inputs["b1"], np.float32)
    Wk2 = np.asarray(inputs["Wk2"], np.float32)
    Wr2 = np.asarray(inputs["Wr2"], np.float32)
    b2 = np.asarray(inputs["b2"], np.float32)
    Wmu = np.asarray(inputs["Wmu"], np.float32)
    bmu = np.asarray(inputs["bmu"], np.float32)
    Wsig = np.asarray(inputs["Wsig"], np.float32)
    bsig = np.asarray(inputs["bsig"], np.float32)

    e0tab = emb0
    e1tab = emb1

    # xT partition order: 0-31 emb0 dims, 32-47 emb1 dims, 48-55 x_cont, 56 ones
    w1 = np.zeros((128, 1 + KC, G4), bf)
    w1[0:E0, 0, :] = Wk1[F:F + E0, :].astype(bf)
    w1[E0:E0 + E1, 0, :] = Wk1[F + E0:DIN, :].astype(bf)
    w1[E0 + E1:E0 + E1 + F, 0, :] = Wk1[0:F, :].astype(bf)
    w1[DIN, 0, :] = b1.astype(bf)
    for c in range(KC):
        w1[:, 1 + c, :] = Wr1[c * 128:(c + 1) * 128, :].astype(bf)
    wk2 = np.zeros((128, KC, G4), bf)
    w2 = np.zeros((128, KC, G4), bf)
    wms = np.zeros((128, KC, 2), bf)
    for c in range(KC):
        wk2[:, c, :] = Wk2[c * 128:(c + 1) * 128, :].astype(bf)
        w2[:, c, :] = Wr2[c * 128:(c + 1) * 128, :].astype(bf)
        wms[:, c, 0] = Wmu[c * 128:(c + 1) * 128, 0].astype(bf)
        wms[:, c, 1] = Wsig[c * 128:(c + 1) * 128, 0].astype(bf)
    b2v = b2.reshape(1, G4)
    bms = np.array([[float(bmu.reshape(-1)[0]), float(bsig.reshape(-1)[0])]],
                   np.float32)

    MT = R // 128

    def wrap_idx(cat):  # [BSH, T] -> (t,b) rows -> [128, MT] int32
        lin = np.ascontiguousarray(cat.T).reshape(-1)       # (t, b) order
        return np.ascontiguousarray(lin.reshape(MT, 128).T.astype(np.int32))

    in_maps = []
    for cidx in range(NC_N):
        sl = slice(cidx * BSH, (cidx + 1) * BSH)
        xcs = xc[sl]                                        # [32, 192, 8]
        rows = xcs.transpose(1, 0, 2).reshape(R, F)      # (t,b) rows
        xcr = np.ascontiguousarray(
            rows.reshape(MT, 128, F).transpose(1, 0, 2).astype(np.float32))
        in_maps.append({
            "xcr": xcr,
            "idx0": wrap_idx(cat0[sl]),
            "idx1": wrap_idx(cat1[sl]),
            "e0tab": e0tab, "e1tab": e1tab,
            "w1": w1, "wk2": wk2, "w2": w2, "b2v": b2v,
            "wms": wms, "bms": bms,
        })
    return in_maps


_RUN_KWARGS = {}   # test harness may set e.g. {"trace": True} for profiling
_LAST_RESULT = []


def kernel(**inputs):
    from concourse.bass_utils import run_bass_kernel_spmd
    in_maps = _marshal(inputs)
    nc = build_nc()
    res = run_bass_kernel_spmd(nc, in_maps, core_ids=list(range(NC_N)),
                               **_RUN_KWARGS)
    _LAST_RESULT.clear()
    _LAST_RESULT.append(res)
    mu = np.concatenate([r["mu"] for r in res.results], axis=0)      # [256, 192]
    sg = np.concatenate([r["sigma"] for r in res.results], axis=0)
    return (mu.reshape(B, T, 1).astype(np.float32),
            sg.reshape(B, T, 1).astype(np.float32))



# revision 15
# speedup vs baseline: 3.2293x; 3.2293x over previous
"""DeepAR (2-layer LSTM, H=512) Trainium2 Bass kernel, 8-core data-parallel.

Model (see reference): x = concat(x_cont, emb0[cat0], emb1[cat1]) [B,T,56]
  -> LSTM(512) -> LSTM(512) -> mu = h@Wmu+bmu ; sigma = softplus(h@Wsig+bsig)

Sharding: batch B=256 split across 8 cores (32 rows each); params replicated.

Per-core device program — transposed-gates formulation. All recurrent
matmuls put the WEIGHTS in the stationary operand and stream h^T, so each
matmul's moving dim is just the 32-row batch:

  gates^T [2048, 32] = sum_c Wr[c]^T-tiles @ h^T_c  (+ [Wk;b] @ [x^T;1])

The PE streams 16 Mtiles x (1 xz + 4 Wr) x 32 rows for L1 and
16 x (4 Wk2 + 4 Wr2) x 32 (+512-row b2 inject) for L2 — ~7.2K rows/step
vs ~21.5K for the batch-major formulation, and h^T is produced directly
by the elementwise tail (no per-step PE transposes).

  - gate tile order [i | f | o | g]: one fused Sigmoid over 12 tiles +
    one Tanh over 4; cell update and h = o*tanh(c) on DVE in bf16
  - both layers interleaved in one scan (L2 runs 1 step behind L1);
    PE order per macro step: Wr1[t], b2/Wk2/Wr2[t-1], head[t-1],
    xzb1[t+1] — the L1 ACT/DVE tail hides under the L2 matmuls
  - head: lhsT = h2^T chunk (stationary), rhs = Wms [128, 2] — 8 rows
    per step, accumulated 16 steps per PSUM bank, staged batch-major
  - embeddings gathered up-front on the Pool queue (96 single-index
    indirect DMAs; multi-index indirect DMA corrupts SBUF on HW), with
    the x^T PE transposes pipelined 2 tiles ahead of the scan
  - all Exp/Ln (softplus) deferred to one epilogue pass -> no ACT
    table swaps inside the scan
"""

import numpy as np
import ml_dtypes

import concourse.bass as bass
import concourse.mybir as mybir
import concourse.tile as tile
from concourse import bacc
from concourse.masks import make_identity

F32 = mybir.dt.float32
BF16 = mybir.dt.bfloat16
I32 = mybir.dt.int32

B, T, F = 256, 192, 8
CARD0, CARD1 = 1000, 100
E0, E1 = 32, 16
H = 512
DIN = F + E0 + E1          # 56
G4 = 4 * H                 # 2048
NC_N = 8                   # cores
BSH = B // NC_N            # 32 batch rows per core
R = T * BSH                # 6144 (t,b)-ordered rows per core
KC = H // 128              # 4 recurrent K-chunks
NM = G4 // 128             # 16 gate-column Mtiles
MT = R // 128              # 48 x^T column tiles
R1 = 4                     # h1^T ring depth
R2 = 8                     # h2^T ring depth
A = mybir.ActivationFunctionType

# gate slot order [i | f | o | g] (i: slots 0-3, f: 4-7, o: 8-11, g: 12-15)
_GATE_BASE = [0, H, 3 * H, 2 * H]   # orig col base per slot-group, z=[i|f|g|o]


def _colperm():
    """P[m*128+p] = original G4 column of (Mtile m, partition p)."""
    P = np.empty(G4, np.int64)
    for m in range(NM):
        base = _GATE_BASE[m // 4]
        chunk = m % 4
        P[m * 128:(m + 1) * 128] = base + chunk * 128 + np.arange(128)
    return P


_NC_CACHE = {}


def build_nc(upto="all"):
    if upto in _NC_CACHE:
        return _NC_CACHE[upto]
    nc = bacc.Bacc("TRN2", num_devices=NC_N)

    # ---------------- DRAM I/O ----------------
    idx0_d = nc.dram_tensor("idx0", [128, MT], I32, kind="ExternalInput")
    idx1_d = nc.dram_tensor("idx1", [128, MT], I32, kind="ExternalInput")
    e0t_d = nc.dram_tensor("e0tab", [CARD0, E0], F32, kind="ExternalInput")
    e1t_d = nc.dram_tensor("e1tab", [CARD1, E1], F32, kind="ExternalInput")
    xcr_d = nc.dram_tensor("xcr", [128, MT, F], F32, kind="ExternalInput")
    w1e_d = nc.dram_tensor("w1e", [64, NM, 128], BF16, kind="ExternalInput")
    wr1_d = nc.dram_tensor("wr1", [128, KC, NM, 128], BF16, kind="ExternalInput")
    wk2_d = nc.dram_tensor("wk2", [128, KC, NM, 128], BF16, kind="ExternalInput")
    wr2_d = nc.dram_tensor("wr2", [128, KC, NM, 128], BF16, kind="ExternalInput")
    b2m_d = nc.dram_tensor("b2m", [16, 128], BF16, kind="ExternalInput")
    wms_d = nc.dram_tensor("wms", [128, KC, 2], BF16, kind="ExternalInput")
    bms_d = nc.dram_tensor("bms", [1, 2], F32, kind="ExternalInput")
    dl16_d = nc.dram_tensor("dl16", [16, NM * BSH], BF16, kind="ExternalInput")

    mu_d = nc.dram_tensor("mu", [BSH, T], F32, kind="ExternalOutput")
    sg_d = nc.dram_tensor("sigma", [BSH, T], F32, kind="ExternalOutput")
    dbg_d = nc.dram_tensor("dbg", [128, T, BSH], F32, kind="ExternalOutput") \
        if upto != "all" else None

    _build_body(nc, upto, locals())
    nc.compile()
    _NC_CACHE[upto] = nc
    return nc


def _build_body(nc, upto, env):
    from contextlib import ExitStack
    idx0_d = env["idx0_d"]; idx1_d = env["idx1_d"]; xcr_d = env["xcr_d"]
    e0t_d = env["e0t_d"]; e1t_d = env["e1t_d"]; w1e_d = env["w1e_d"]
    wr1_d = env["wr1_d"]; wk2_d = env["wk2_d"]; wr2_d = env["wr2_d"]
    b2m_d = env["b2m_d"]; wms_d = env["wms_d"]; bms_d = env["bms_d"]
    dl16_d = env["dl16_d"]
    mu_d = env["mu_d"]; sg_d = env["sg_d"]; dbg_d = env["dbg_d"]

    with tile.TileContext(nc) as tc, ExitStack() as top:
        singles = top.enter_context(tc.tile_pool(name="singles", bufs=1))

        # ---------------- weights / constants to SBUF ----------------
        w1e = singles.tile([64, NM, 128], BF16)
        nc.sync.dma_start(out=w1e[:], in_=w1e_d[:])
        wr1 = singles.tile([128, KC, NM, 128], BF16)
        nc.sync.dma_start(out=wr1[:], in_=wr1_d[:])
        wk2 = singles.tile([128, KC, NM, 128], BF16)
        nc.sync.dma_start(out=wk2[:], in_=wk2_d[:])
        wr2 = singles.tile([128, KC, NM, 128], BF16)
        nc.sync.dma_start(out=wr2[:], in_=wr2_d[:])
        b2m = singles.tile([16, 128], BF16)
        nc.sync.dma_start(out=b2m[:], in_=b2m_d[:])
        wms = singles.tile([128, KC, 2], BF16)
        nc.sync.dma_start(out=wms[:], in_=wms_d[:])
        bms = singles.tile([1, 2], F32)
        nc.sync.dma_start(out=bms[:], in_=bms_d[:])
        # bmu/bsig broadcast to 32 partitions for the epilogue
        bmu32 = singles.tile([BSH, 1], F32)
        nc.sync.dma_start(
            out=bmu32[:],
            in_=bass.AP(tensor=bms_d[:].tensor, offset=0, ap=[[0, BSH], [1, 1]]))
        bsg32 = singles.tile([BSH, 1], F32)
        nc.sync.dma_start(
            out=bsg32[:],
            in_=bass.AP(tensor=bms_d[:].tensor, offset=1, ap=[[0, BSH], [1, 1]]))

        ident_f32 = singles.tile([128, 128], F32)
        make_identity(nc, ident_f32[:])

        # delta16[k, (m, j)] = 1 if k == m else 0 — b2-inject rhs
        delta16 = singles.tile([16, NM, BSH], BF16)
        nc.sync.dma_start(out=delta16[:], in_=dl16_d[:])

        # h^T rings, cell states (bf16), head staging
        h1h = singles.tile([128, R1, KC, BSH], BF16)
        h2h = singles.tile([128, R2, KC, BSH], BF16)
        c1 = singles.tile([128, KC, BSH], BF16)
        c2 = singles.tile([128, KC, BSH], BF16)
        stage = singles.tile([BSH, T, 2], F32)

        # ---------------- phase 1: gather + x^T build ----------------
        gp = top.enter_context(tc.tile_pool(name="gather", bufs=1))
        idx0_sb = gp.tile([128, MT], I32)
        nc.sync.dma_start(out=idx0_sb[:], in_=idx0_d[:])
        idx1_sb = gp.tile([128, MT], I32)
        nc.sync.dma_start(out=idx1_sb[:], in_=idx1_d[:])
        # assembled rows: [p, m, 64] = [e0 | e1 | xc | ones(+pad)]
        asm = gp.tile([128, MT, 64], F32)
        nc.vector.memset(asm[:], 1.0)
        xcb = gp.tile([128, MT, F], F32)
        nc.sync.dma_start(out=xcb[:], in_=xcr_d[:])
        nc.vector.tensor_copy(asm[:, :, E0 + E1:DIN], xcb[:])
        for m in range(MT):
            nc.gpsimd.indirect_dma_start(
                out=asm[:, m, 0:E0], out_offset=None, in_=e0t_d[:],
                in_offset=bass.IndirectOffsetOnAxis(
                    ap=idx0_sb[:, m:m + 1], axis=0))
            nc.gpsimd.indirect_dma_start(
                out=asm[:, m, E0:E0 + E1], out_offset=None, in_=e1t_d[:],
                in_offset=bass.IndirectOffsetOnAxis(
                    ap=idx1_sb[:, m:m + 1], axis=0))

        xT = top.enter_context(tc.tile_pool(name="xtp", bufs=1)).tile([64, R], BF16)
        ptr = top.enter_context(tc.tile_pool(name="gtr", bufs=2, space="PSUM"))

        def xT_tile(m):
            ps = ptr.tile([64, 128], F32, tag="tr")
            nc.tensor.transpose(ps[:], asm[:, m, :], ident_f32[:])
            nc.vector.tensor_copy(xT[:, 128 * m:128 * (m + 1)], ps[:])

        xT_tile(0)
        xT_tile(1)
        xT_done = 2

        if upto == "xT":
            for m in range(2, MT):
                xT_tile(m)
            with tc.tile_pool(name="dbgp", bufs=1) as dp:
                dbg_sb = dp.tile([64, R], F32)
                nc.vector.tensor_copy(dbg_sb[:], xT[:])
                dv = bass.AP(tensor=dbg_d[:].tensor, offset=0,
                             ap=[[T * BSH, 64], [1, R]])
                nc.sync.dma_start(out=dv, in_=dbg_sb[:])
            return
        dbg_sb = None
        if upto in ("h1", "h2"):
            dbg_sb = singles.tile([128, T, BSH], F32)

        # ---------------- main interleaved scan ----------------
        ew = top.enter_context(tc.tile_pool(name="ew", bufs=2))
        psg1 = top.enter_context(tc.tile_pool(name="psg1", bufs=2, space="PSUM"))
        psg2 = top.enter_context(tc.tile_pool(name="psg2", bufs=2, space="PSUM"))
        psh = top.enter_context(tc.tile_pool(name="psh", bufs=2, space="PSUM"))

        pg1 = {}
        pg2 = {}
        psH = [None]

        def xzb1(t):
            # input projection + b1 for step t (starts the psum group)
            pg = psg1.tile([128, NM, BSH], F32, tag="pg1")
            pg1[t] = pg
            # start=True zeroes the whole 2KB zero region (= this bank):
            # only the first matmul into the bank may set it
            for m in range(NM):
                nc.tensor.matmul(pg[:, m, :], w1e[0:57, m, :],
                                 xT[0:57, t * BSH:(t + 1) * BSH],
                                 start=(m == 0),
                                 stop=(t == 0 and m == NM - 1),
                                 skip_group_check=True)

        def wr_mm(pg, w, hring, rslot, t0):
            for m in range(NM):
                for c in range(KC):
                    nc.tensor.matmul(pg[:, m, :], w[:, c, m, :],
                                     hring[:, rslot, c, :],
                                     start=False,
                                     stop=(t0 and m == NM - 1 and c == KC - 1),
                                     skip_group_check=True)

        def tail(t, pg, cst, hring, rslot):
            # gates^T [128, 16, 32] -> h^T ring slot; [i | f | o | g]
            sig = ew.tile([128, 12, BSH], BF16, tag="sig")
            nc.scalar.activation(sig[:], pg[:, 0:12, :], A.Sigmoid)
            tng = ew.tile([128, KC, BSH], BF16, tag="tng")
            nc.scalar.activation(tng[:], pg[:, 12:16, :], A.Tanh)
            ig = ew.tile([128, KC, BSH], BF16, tag="ig")
            nc.vector.tensor_mul(ig[:], sig[:, 0:KC, :], tng[:])
            if t == 0:
                nc.vector.tensor_copy(cst[:], ig[:])
            else:
                fc = ew.tile([128, KC, BSH], BF16, tag="fc")
                nc.vector.tensor_mul(fc[:], sig[:, KC:2 * KC, :], cst[:])
                nc.vector.tensor_add(cst[:], fc[:], ig[:])
            tnc = ew.tile([128, KC, BSH], BF16, tag="tnc")
            nc.scalar.activation(tnc[:], cst[:], A.Tanh)
            nc.vector.tensor_mul(hring[:, rslot, :, :],
                                 sig[:, 2 * KC:3 * KC, :], tnc[:])

        def head(s):
            # mu/sigma for step s: out [32, 2] <- sum_c h2^T_c(stat) @ wms_c
            if s % 16 == 0:
                psH[0] = psh.tile([BSH, 16, 2], F32, tag="psH", name="psH")
            for c in range(KC):
                nc.tensor.matmul(psH[0][:, s % 16, :], h2h[:, s % R2, c, :],
                                 wms[:, c, :],
                                 start=(c == 0 and s % 16 == 0),
                                 stop=(c == KC - 1 and s % 16 == 15),
                                 skip_group_check=True)
            if s % 16 == 15:
                nc.vector.tensor_copy(stage[:, s - 15:s + 1, :], psH[0][:])

        # macro step t: Wr1[t] -> L1 tail[t] -> L2[t-1] -> head[t-1]
        # -> xzb1[t+1] (+ pipelined x^T transposes)
        xzb1(0)
        for t in range(T + 1):
            if t < T:
                if t > 0:
                    wr_mm(pg1[t], wr1, h1h, (t - 1) % R1, True)
                tail(t, pg1.pop(t), c1, h1h, t % R1)
            s = t - 1
            if 0 <= s < T:
                pg = psg2.tile([128, NM, BSH], F32, tag="pg2")
                nc.tensor.matmul(pg[:], b2m[:], delta16[:],
                                 start=True, stop=False, skip_group_check=True)
                wr_mm(pg, wk2, h1h, s % R1, s == 0)
                if s > 0:
                    wr_mm(pg, wr2, h2h, (s - 1) % R2, True)
                tail(s, pg, c2, h2h, s % R2)
                head(s)
            if t + 1 < T:
                xzb1(t + 1)
            while xT_done * 4 < t + 10 and xT_done < MT:
                xT_tile(xT_done)
                xT_done += 1
            if upto == "h1" and t < T:
                nc.vector.tensor_copy(dbg_sb[:, t, :], h1h[:, t % R1, 0, :])
            if upto == "h2" and 0 <= s < T:
                nc.vector.tensor_copy(dbg_sb[:, s, :], h2h[:, s % R2, 0, :])

        # ---------------- epilogue: bias, softplus, DMA out ----------------
        ep = top.enter_context(tc.tile_pool(name="ep", bufs=1))
        mu_sb = ep.tile([BSH, T], F32)
        nc.vector.tensor_scalar_add(mu_sb[:], stage[:, :, 0], bmu32[:])
        sg_e = ep.tile([BSH, T], F32)
        nc.scalar.activation(sg_e[:], stage[:, :, 1], A.Exp, bias=bsg32[:])
        sg_sb = ep.tile([BSH, T], F32)
        nc.scalar.activation(sg_sb[:], sg_e[:], A.Ln, bias=1.0)
        nc.sync.dma_start(out=mu_d[:], in_=mu_sb[:])
        nc.sync.dma_start(out=sg_d[:], in_=sg_sb[:])
        if upto in ("h1", "h2"):
            nc.sync.dma_start(out=dbg_d[:], in_=dbg_sb[:])

    return nc


def _marshal(inputs):
    """Host-side shard/layout marshalling (no compute beyond dtype cast/pad)."""
    bf = ml_dtypes.bfloat16
    xc = np.ascontiguousarray(np.asarray(inputs["x_cont"], np.float32))
    cat0 = np.asarray(inputs["cat0"]).astype(np.int32)
    cat1 = np.asarray(inputs["cat1"]).astype(np.int32)
    emb0 = np.asarray(inputs["emb0"], np.float32)
    emb1 = np.asarray(inputs["emb1"], np.float32)
    Wk1 = np.asarray(inputs["Wk1"], np.float32)
    Wr1 = np.asarray(inputs["Wr1"], np.float32)
    b1 = np.asarray(inputs["b1"], np.float32)
    Wk2 = np.asarray(inputs["Wk2"], np.float32)
    Wr2 = np.asarray(inputs["Wr2"], np.float32)
    b2 = np.asarray(inputs["b2"], np.float32)
    Wmu = np.asarray(inputs["Wmu"], np.float32)
    bmu = np.asarray(inputs["bmu"], np.float32)
    Wsig = np.asarray(inputs["Wsig"], np.float32)
    bsig = np.asarray(inputs["bsig"], np.float32)

    P = _colperm()

    # xT partition order: 0-31 emb0 dims, 32-47 emb1 dims, 48-55 x_cont, 56 ones
    wk1_rows = np.concatenate([Wk1[F:F + E0], Wk1[F + E0:DIN], Wk1[0:F],
                               b1.reshape(1, G4)], axis=0)      # [57, G4]
    w1e = np.zeros((64, NM, 128), bf)
    w1e[0:57] = wk1_rows[:, P].reshape(57, NM, 128).astype(bf)

    def kperm(W):  # [512, G4] -> [128(k), KC, NM, 128]
        Wp = W[:, P].reshape(KC, 128, NM, 128)
        return np.ascontiguousarray(Wp.transpose(1, 0, 2, 3)).astype(bf)

    wr1 = kperm(Wr1)
    wk2 = kperm(Wk2)
    wr2 = kperm(Wr2)
    b2m = np.ascontiguousarray(b2[P].reshape(16, 128)).astype(bf)
    wms = np.zeros((128, KC, 2), bf)
    for c in range(KC):
        wms[:, c, 0] = Wmu[c * 128:(c + 1) * 128, 0].astype(bf)
        wms[:, c, 1] = Wsig[c * 128:(c + 1) * 128, 0].astype(bf)
    bms = np.array([[float(bmu.reshape(-1)[0]), float(bsig.reshape(-1)[0])]],
                   np.float32)
    dl16 = np.kron(np.eye(16, dtype=np.float32),
                   np.ones((1, BSH), np.float32)).astype(bf)  # [16, 512]

    def wrap_idx(cat):  # [BSH, T] -> (t,b) rows -> [128, MT] int32
        lin = np.ascontiguousarray(cat.T).reshape(-1)       # (t, b) order
        return np.ascontiguousarray(lin.reshape(MT, 128).T.astype(np.int32))

    in_maps = []
    for cidx in range(NC_N):
        sl = slice(cidx * BSH, (cidx + 1) * BSH)
        xcs = xc[sl]                                        # [32, 192, 8]
        rows = xcs.transpose(1, 0, 2).reshape(R, F)         # (t,b) rows
        xcr = np.ascontiguousarray(
            rows.reshape(MT, 128, F).transpose(1, 0, 2).astype(np.float32))
        in_maps.append({
            "xcr": xcr,
            "idx0": wrap_idx(cat0[sl]),
            "idx1": wrap_idx(cat1[sl]),
            "e0tab": emb0, "e1tab": emb1,
            "w1e": w1e, "wr1": wr1, "wk2": wk2, "wr2": wr2,
            "b2m": b2m, "wms": wms, "bms": bms, "dl16": dl16,
        })
    return in_maps


_RUN_KWARGS = {}   # test harness may set e.g. {"trace": True} for profiling
_LAST_RESULT = []


def kernel(**inputs):
    from concourse.bass_utils import run_bass_kernel_spmd
    in_maps = _marshal(inputs)
    nc = build_nc()
    res = run_bass_kernel_spmd(nc, in_maps, core_ids=list(range(NC_N)),
                               **_RUN_KWARGS)
    _LAST_RESULT.clear()
    _LAST_RESULT.append(res)
    mu = np.concatenate([r["mu"] for r in res.results], axis=0)      # [256, 192]
    sg = np.concatenate([r["sigma"] for r in res.results], axis=0)
    return (mu.reshape(B, T, 1).astype(np.float32),
            sg.reshape(B, T, 1).astype(np.float32))
